# revision 2
# baseline (speedup 1.0000x reference)
"""Trainium2 Bass kernel for nn_DetectSpikes (spatiotemporal NMS spike detection).

kernel(traces [150000,384] f32, channel_locations [384,2] f32) ->
(times int64 [100000], chans int32 [100000]) matching the reference exactly.

Detection rule (x = -traces): (n, m) is a detection iff x >= 3.0, time margin
20, and x >= every x[n', m'] with |n'-n| <= 15, m' adjacent (radius 100).

Device (8 cores, time-sharded with halo, SPMD), per core:
  - Input is bf16(x) [t_loc, 384]; PE transposes [time, chan] tiles into
    [chan, time] bf16 PSUM via identity matmuls.
  - 16-sample block maxima B16 per channel via a max tree: ACT copies the
    even half PSUM->SBUF (DVE may read only one PSUM operand), DVE does
    3 further pairwise-max levels (bf16 SBUF, 2x DVE mode).
  - B16 [chan, n_blocks] bf16 is the only output (exact maxima of the bf16
    inputs - no other approximation on device).
Host: exact resolution from B16 + raw f32 traces.
  - bf16 RTN is monotone, so any x' > xv has bf16(x') >= bf16(xv); block
    covers therefore identify every possibly-dominating neighbor channel
    ("live" set), and candidates are exactly re-checked against raw f32
    windows.  Output is exact for ANY input.
"""

import time

import numpy as np
import ml_dtypes

import concourse.bass as bass
import concourse.tile as tile
from concourse import bacc, mybir
from concourse.bass_utils import run_bass_kernel_spmd

# ---- problem constants ----
N, M = 150000, 384
TR = 15
THR = 3.0
MARGIN = 20
RADIUS = 100.0
MAX_DET = 100000
NCORES = 8
INT = N // NCORES             # 18750

BLK = 16
CHUNK = 1024
N_FULL = 18                   # full 1024-sample chunks
TAIL = 512                    # one tail chunk
T_LOC = N_FULL * CHUNK + TAIL  # 18944 >= INT + 2*TR
NB = T_LOC // BLK             # 1184 blocks per channel per core

_BF16 = mybir.dt.bfloat16

# output DMA group boundaries (in blocks); last group lands after final chunk
_OUT_GROUPS = [(0, 320), (320, 640), (640, 960), (960, NB)]


def build_program():
    nc = bacc.Bacc(
        "TRN2", target_bir_lowering=False, debug=False, enable_asserts=False,
        num_devices=NCORES,
    )
    xs = nc.dram_tensor("xs", [T_LOC, 384], _BF16, kind="ExternalInput")
    ident = nc.dram_tensor("ident", [128, 128], _BF16, kind="ExternalInput")
    b16_d = nc.dram_tensor("b16", [128, 3 * NB], _BF16, kind="ExternalOutput")

    from contextlib import ExitStack
    with tile.TileContext(nc) as tc, ExitStack() as ctx:
        consts = ctx.enter_context(tc.tile_pool(name="consts", bufs=1))
        persist = ctx.enter_context(tc.tile_pool(name="persist", bufs=1))
        rawp = ctx.enter_context(tc.tile_pool(name="raw", bufs=3))
        psx = ctx.enter_context(tc.tile_pool(name="psx", bufs=2, space="PSUM"))
        y8hp = ctx.enter_context(tc.tile_pool(name="y8h", bufs=2))
        y8p = ctx.enter_context(tc.tile_pool(name="y8", bufs=2))
        y4p = ctx.enter_context(tc.tile_pool(name="y4", bufs=2))
        y2p = ctx.enter_context(tc.tile_pool(name="y2", bufs=2))

        it = consts.tile([128, 128], _BF16, tag="ident")
        nc.sync.dma_start(it[:], ident.ap()[:, :])

        ball = persist.tile([128, 3 * NB], _BF16, tag="ball")
        ballv = ball[:].rearrange("p (s k) -> p s k", s=3)

        out_done = 0
        for i in range(N_FULL + 1):
            ch = CHUNK if i < N_FULL else TAIL
            nj = ch // 128
            nb = ch // BLK
            r0 = i * CHUNK
            boff = r0 // BLK

            raw = rawp.tile([128, 8 * 384], _BF16, tag="raw")
            raw_v = raw[:].rearrange("p (j c) -> p j c", c=384)
            src = xs.ap()[r0:r0 + ch, :].rearrange("(j p) c -> p j c", p=128)
            nc.sync.dma_start(raw_v[:, 0:nj], src)

            ps = psx.tile([128, 3 * CHUNK], _BF16, tag="ps")
            for db in range(3):
                for j in range(nj):
                    nc.tensor.matmul(
                        ps[:, CHUNK * db + 128 * j: CHUNK * db + 128 * (j + 1)],
                        raw_v[:, j, 128 * db:128 * (db + 1)],
                        it[:], is_transpose=True, start=True, stop=True,
                    )
            pv = ps[:].rearrange("p (s b h e) -> p s b h e", s=3, b=CHUNK // BLK,
                                 h=2, e=8)
            y8h = y8hp.tile([128, 3 * 512], _BF16, tag="y8h")
            y8hv = y8h[:].rearrange("p (s b e) -> p s b e", s=3, e=8)
            nc.scalar.copy(y8hv[:, :, 0:nb], pv[:, :, 0:nb, 0, :])
            y8 = y8p.tile([128, 3 * 512], _BF16, tag="y8")
            y8v = y8[:].rearrange("p (s b e) -> p s b e", s=3, e=8)
            nc.vector.tensor_tensor(y8v[:, :, 0:nb], pv[:, :, 0:nb, 1, :],
                                    y8hv[:, :, 0:nb], mybir.AluOpType.max)
            y4 = y4p.tile([128, 3 * 256], _BF16, tag="y4")
            y4v = y4[:].rearrange("p (s b e) -> p s b e", s=3, e=4)
            a = y8[:].rearrange("p (s b h e) -> p s b h e", s=3, b=64, h=2, e=4)
            nc.vector.tensor_tensor(y4v[:, :, 0:nb], a[:, :, 0:nb, 0, :],
                                    a[:, :, 0:nb, 1, :], mybir.AluOpType.max)
            y2 = y2p.tile([128, 3 * 128], _BF16, tag="y2")
            y2v = y2[:].rearrange("p (s b e) -> p s b e", s=3, e=2)
            a = y4[:].rearrange("p (s b h e) -> p s b h e", s=3, b=64, h=2, e=2)
            nc.vector.tensor_tensor(y2v[:, :, 0:nb], a[:, :, 0:nb, 0, :],
                                    a[:, :, 0:nb, 1, :], mybir.AluOpType.max)
            a = y2[:].rearrange("p (s b h) -> p s b h", s=3, h=2)
            nc.vector.tensor_tensor(ballv[:, :, boff:boff + nb],
                                    a[:, :, 0:nb, 0], a[:, :, 0:nb, 1],
                                    mybir.AluOpType.max)

            # stream finished block ranges out
            done = boff + nb
            while out_done < len(_OUT_GROUPS) and _OUT_GROUPS[out_done][1] <= done:
                a0, a1 = _OUT_GROUPS[out_done]
                nc.sync.dma_start(
                    b16_d.ap().rearrange("p (s k) -> p s k", s=3)[:, :, a0:a1],
                    ballv[:, :, a0:a1],
                )
                out_done += 1

    nc.compile()
    return nc


# ------------------------ host side ------------------------

def _adjacency(channel_locations):
    locs = np.asarray(channel_locations, np.float32)
    d2 = ((locs[:, None, :] - locs[None, :, :]) ** 2).sum(-1, dtype=np.float32)
    return np.sqrt(d2.astype(np.float32)) <= np.float32(RADIUS)


def _nbr_table(adj):
    deg = adj.sum(0)
    dmax = int(deg.max())
    nbr = np.zeros((M, dmax), np.int32)
    nbr_ok = np.zeros((M, dmax), bool)
    for m in range(M):
        js = np.flatnonzero(adj[:, m])
        nbr[m, : len(js)] = js
        nbr_ok[m, : len(js)] = True
    return nbr, nbr_ok


def _postprocess_core(Bf, xneg, xb, nbr, nbr_ok, start, g0, g1):
    """Bf [384, NB] f32 (bf16 block maxima), xneg full [N,384] f32 = -traces,
    xb full bf16 array.  Interior global rows [g0, g1).  Exact output."""
    # cover of blocks b-1..b+1, edge-clipped
    cov = Bf.copy()
    np.maximum(cov[:, 1:], Bf[:, :-1], out=cov[:, 1:])
    np.maximum(cov[:, :-1], Bf[:, 1:], out=cov[:, :-1])

    hitc, hitk = np.nonzero(Bf >= np.float32(THR - 1.0 / 128))
    if hitc.size == 0:
        return np.empty(0, np.int64), np.empty(0, np.int64)
    tg = (hitk * BLK + start)[:, None] + np.arange(BLK)[None, :]   # [H,16]
    lo = max(g0, MARGIN)
    hi = min(g1, N - MARGIN)
    xv = xneg[tg, hitc[:, None]]
    ok = (xv >= THR) & (tg >= lo) & (tg < hi)
    pi, ri = np.nonzero(ok)
    if pi.size == 0:
        return np.empty(0, np.int64), np.empty(0, np.int64)
    mm = hitc[pi]
    tgs = tg[pi, ri]
    xvs = xv[pi, ri]
    blk = (tgs - start) // BLK
    xvb = xb[tgs, mm].astype(np.float32)

    covn = cov[nbr[mm], blk[:, None]]                  # [P, D]
    live = (covn >= xvb[:, None]) & nbr_ok[mm]
    p2, d2 = np.nonzero(live)
    jj = nbr[mm[p2], d2]
    tt = tgs[p2]
    t0 = np.maximum(tt - TR, 0)
    t1 = np.minimum(tt + TR, N - 1)
    tw = t0[:, None] + np.arange(2 * TR + 1)[None, :]
    np.minimum(tw, t1[:, None], out=tw)
    g = xneg[tw, jj[:, None]].max(1)
    starts = np.unique(p2, return_index=True)[1]       # every cand has self live
    segmax = np.maximum.reduceat(g, starts)
    keep = xvs >= segmax
    mm, tgs = mm[keep], tgs[keep]
    o = np.lexsort((mm, tgs))
    return tgs[o], mm[o].astype(np.int64)


_PROGRAM_CACHE = {}


def kernel(traces, channel_locations):
    traces = np.ascontiguousarray(np.asarray(traces, np.float32))
    xneg = -traces
    xb = xneg.astype(ml_dtypes.bfloat16)
    adj = _adjacency(channel_locations)
    nbr, nbr_ok = _nbr_table(adj)
    identity = np.eye(128, dtype=np.float32).astype(ml_dtypes.bfloat16)

    if "full" not in _PROGRAM_CACHE:
        _PROGRAM_CACHE["full"] = build_program()
    nc = _PROGRAM_CACHE["full"]

    starts = [min(max(c * INT - TR, 0), N - T_LOC) for c in range(NCORES)]
    in_maps = [{
        "xs": np.ascontiguousarray(xb[starts[c]: starts[c] + T_LOC]),
        "ident": identity,
    } for c in range(NCORES)]
    try:
        res = run_bass_kernel_spmd(nc, in_maps, list(range(NCORES)))
    except Exception:
        time.sleep(2.0)
        res = run_bass_kernel_spmd(nc, in_maps, list(range(NCORES)))
    results = res.results

    all_t, all_c = [], []
    for c in range(NCORES):
        out = np.asarray(results[c]["b16"]).reshape(128, 3, NB)
        Bf = out.transpose(1, 0, 2).reshape(384, NB).astype(np.float32)
        t_, c_ = _postprocess_core(Bf, xneg, xb, nbr, nbr_ok, starts[c],
                                   c * INT, (c + 1) * INT)
        all_t.append(t_)
        all_c.append(c_)

    times = np.concatenate(all_t) if all_t else np.empty(0, np.int64)
    chans = np.concatenate(all_c) if all_c else np.empty(0, np.int64)
    times, chans = times[:MAX_DET], chans[:MAX_DET]
    out_t = np.full(MAX_DET, -1, np.int64)
    out_c = np.full(MAX_DET, -1, np.int32)
    out_t[: times.size] = times
    out_c[: chans.size] = chans
    return out_t, out_c


# revision 17
# speedup vs baseline: 1.1097x; 1.1097x over previous
"""Trainium2 Bass kernel for nn_DetectSpikes (spatiotemporal NMS spike detection).

kernel(traces [150000,384] f32, channel_locations [384,2] f32) ->
(times int64 [100000], chans int32 [100000]) matching the reference exactly.

Detection rule (x = -traces): (n, m) is a detection iff x >= 3.0, time margin
20, and x >= every x[n', m'] with |n'-n| <= 15, m' adjacent (radius 100).

Device (8 cores, time-sharded with halo, SPMD), per core:
  - Input is bf16(x) [t_loc, 384]; PE transposes [time, chan] tiles into
    [chan, time] bf16 PSUM via identity matmuls.
  - 16-sample block maxima B16 per channel via a max tree: ACT copies the
    even half PSUM->SBUF (DVE may read only one PSUM operand), DVE does
    3 further pairwise-max levels (bf16 SBUF, 2x DVE mode).
  - B16 [chan, n_blocks] bf16 is the only output (exact maxima of the bf16
    inputs - no other approximation on device).
Host: exact resolution from B16 + raw f32 traces.
  - bf16 RTN is monotone, so any x' > xv has bf16(x') >= bf16(xv); block
    covers therefore identify every possibly-dominating neighbor channel
    ("live" set), and candidates are exactly re-checked against raw f32
    windows.  Output is exact for ANY input.
"""

import time

import numpy as np
import ml_dtypes

import concourse.bass as bass
import concourse.tile as tile
from concourse import bacc, mybir
from concourse.bass_utils import run_bass_kernel_spmd

# ---- problem constants ----
N, M = 150000, 384
TR = 15
THR = 3.0
MARGIN = 20
RADIUS = 100.0
MAX_DET = 100000
NCORES = 8
INT = N // NCORES             # 18750

BLK = 16
CHUNK = 1024
# full chunks amortize fixed op costs, small tail chunks drain fast
_CHUNKS = [1024] * 18 + [256, 128]
T_LOC = sum(_CHUNKS)          # 18816 >= INT + 2*TR
NB = T_LOC // BLK             # 1176 blocks per channel per core

_BF16 = mybir.dt.bfloat16

# output DMA batching (blocks); the last (tiny) group drains on the ACT queue
_OUT_EDGES = [1024, 1168, NB]
_N_WARM = 26


def build_program():
    nc = bacc.Bacc(
        "TRN2", target_bir_lowering=False, debug=False, enable_asserts=False,
        num_devices=NCORES,
    )
    xs = nc.dram_tensor("xs", [T_LOC, 384], _BF16, kind="ExternalInput")
    ident = nc.dram_tensor("ident", [128, 128], _BF16, kind="ExternalInput")
    b16_d = nc.dram_tensor("b16", [128, 3 * NB], _BF16, kind="ExternalOutput")

    from contextlib import ExitStack
    with tile.TileContext(nc) as tc, ExitStack() as ctx:
        consts = ctx.enter_context(tc.tile_pool(name="consts", bufs=1))
        persist = ctx.enter_context(tc.tile_pool(name="persist", bufs=1))
        rawp = ctx.enter_context(tc.tile_pool(name="raw", bufs=4))
        psx = ctx.enter_context(tc.tile_pool(name="psx", bufs=2, space="PSUM"))
        y8hp = ctx.enter_context(tc.tile_pool(name="y8h", bufs=2))
        y8p = ctx.enter_context(tc.tile_pool(name="y8", bufs=2))
        y4p = ctx.enter_context(tc.tile_pool(name="y4", bufs=2))
        y2p = ctx.enter_context(tc.tile_pool(name="y2", bufs=2))

        # warm the PE pstate ramp while the first input DMAs are in flight;
        # a memset scratch operand avoids any DMA dependency
        wz = consts.tile([128, 128], _BF16, tag="warmz")
        nc.vector.memset(wz[:], 0.0)
        warm = psx.tile([128, 128], _BF16, tag="warm")
        for _ in range(_N_WARM):
            nc.tensor.matmul(warm[:], wz[:], wz[:], is_transpose=True,
                             start=True, stop=True)

        it = consts.tile([128, 128], _BF16, tag="ident")

        ball = persist.tile([128, 3 * NB], _BF16, tag="ball")
        ballv = ball[:].rearrange("p (s k) -> p s k", s=3)
        out_done = 0
        r0 = 0
        for i, ch in enumerate(_CHUNKS):
            nj = ch // 128
            nb = ch // BLK
            boff = r0 // BLK

            raw = rawp.tile([128, 8 * 384], _BF16, tag="raw")
            raw_v = raw[:].rearrange("p (j c) -> p j c", c=384)
            src = xs.ap()[r0:r0 + ch, :].rearrange("(j p) c -> p j c", p=128)
            nc.sync.dma_start(raw_v[:, 0:nj], src)
            if i == 0:
                # ident rides behind the first input transfer
                nc.sync.dma_start(it[:], ident.ap()[:, :])

            ps = psx.tile([128, 3 * CHUNK], _BF16, tag="ps")
            for db in range(3):
                for j in range(nj):
                    nc.tensor.matmul(
                        ps[:, CHUNK * db + 128 * j: CHUNK * db + 128 * (j + 1)],
                        raw_v[:, j, 128 * db:128 * (db + 1)],
                        it[:], is_transpose=True, start=True, stop=True,
                    )
            pv = ps[:].rearrange("p (s b h e) -> p s b h e", s=3, b=CHUNK // BLK,
                                 h=2, e=8)
            y8h = y8hp.tile([128, 3 * 512], _BF16, tag="y8h")
            y8hv = y8h[:].rearrange("p (s b e) -> p s b e", s=3, e=8)
            nc.scalar.copy(y8hv[:, :, 0:nb], pv[:, :, 0:nb, 0, :])
            y8 = y8p.tile([128, 3 * 512], _BF16, tag="y8")
            y8v = y8[:].rearrange("p (s b e) -> p s b e", s=3, e=8)
            nc.vector.tensor_tensor(y8v[:, :, 0:nb], pv[:, :, 0:nb, 1, :],
                                    y8hv[:, :, 0:nb], mybir.AluOpType.max)
            y4 = y4p.tile([128, 3 * 256], _BF16, tag="y4")
            y4v = y4[:].rearrange("p (s b e) -> p s b e", s=3, e=4)
            a = y8[:].rearrange("p (s b h e) -> p s b h e", s=3, b=64, h=2, e=4)
            nc.vector.tensor_tensor(y4v[:, :, 0:nb], a[:, :, 0:nb, 0, :],
                                    a[:, :, 0:nb, 1, :], mybir.AluOpType.max)
            y2 = y2p.tile([128, 3 * 128], _BF16, tag="y2")
            y2v = y2[:].rearrange("p (s b e) -> p s b e", s=3, e=2)
            a = y4[:].rearrange("p (s b h e) -> p s b h e", s=3, b=64, h=2, e=2)
            nc.vector.tensor_tensor(y2v[:, :, 0:nb], a[:, :, 0:nb, 0, :],
                                    a[:, :, 0:nb, 1, :], mybir.AluOpType.max)
            a = y2[:].rearrange("p (s b h) -> p s b h", s=3, h=2)
            nc.vector.tensor_tensor(ballv[:, :, boff:boff + nb],
                                    a[:, :, 0:nb, 0], a[:, :, 0:nb, 1],
                                    mybir.AluOpType.max)

            # stream finished block groups out on the otherwise-idle Pool
            # queue so neither the SP input stream nor the ACT copy stream
            # ever waits behind an output trigger
            done = boff + nb
            while out_done < len(_OUT_EDGES) and _OUT_EDGES[out_done] <= done:
                a0 = _OUT_EDGES[out_done - 1] if out_done else 0
                a1 = _OUT_EDGES[out_done]
                eng = nc.scalar if a1 == NB else nc.gpsimd
                eng.dma_start(
                    b16_d.ap().rearrange("p (s k) -> p s k", s=3)[:, :, a0:a1],
                    ballv[:, :, a0:a1],
                )
                out_done += 1
            r0 += ch

    nc.compile()
    return nc


# ------------------------ host side ------------------------

def _adjacency(channel_locations):
    locs = np.asarray(channel_locations, np.float32)
    d2 = ((locs[:, None, :] - locs[None, :, :]) ** 2).sum(-1, dtype=np.float32)
    return np.sqrt(d2.astype(np.float32)) <= np.float32(RADIUS)


def _nbr_table(adj):
    deg = adj.sum(0)
    dmax = int(deg.max())
    nbr = np.zeros((M, dmax), np.int32)
    nbr_ok = np.zeros((M, dmax), bool)
    for m in range(M):
        js = np.flatnonzero(adj[:, m])
        nbr[m, : len(js)] = js
        nbr_ok[m, : len(js)] = True
    return nbr, nbr_ok


def _postprocess_core(Bf, xneg, xb, nbr, nbr_ok, start, g0, g1):
    """Bf [384, NB] f32 (bf16 block maxima), xneg full [N,384] f32 = -traces,
    xb full bf16 array.  Interior global rows [g0, g1).  Exact output."""
    # cover of blocks b-1..b+1, edge-clipped
    cov = Bf.copy()
    np.maximum(cov[:, 1:], Bf[:, :-1], out=cov[:, 1:])
    np.maximum(cov[:, :-1], Bf[:, 1:], out=cov[:, :-1])

    hitc, hitk = np.nonzero(Bf >= np.float32(THR - 1.0 / 128))
    if hitc.size == 0:
        return np.empty(0, np.int64), np.empty(0, np.int64)
    tg = (hitk * BLK + start)[:, None] + np.arange(BLK)[None, :]   # [H,16]
    lo = max(g0, MARGIN)
    hi = min(g1, N - MARGIN)
    xv = xneg[tg, hitc[:, None]]
    ok = (xv >= THR) & (tg >= lo) & (tg < hi)
    pi, ri = np.nonzero(ok)
    if pi.size == 0:
        return np.empty(0, np.int64), np.empty(0, np.int64)
    mm = hitc[pi]
    tgs = tg[pi, ri]
    xvs = xv[pi, ri]
    blk = (tgs - start) // BLK
    xvb = xb[tgs, mm].astype(np.float32)

    covn = cov[nbr[mm], blk[:, None]]                  # [P, D]
    live = (covn >= xvb[:, None]) & nbr_ok[mm]
    p2, d2 = np.nonzero(live)
    jj = nbr[mm[p2], d2]
    tt = tgs[p2]
    t0 = np.maximum(tt - TR, 0)
    t1 = np.minimum(tt + TR, N - 1)
    tw = t0[:, None] + np.arange(2 * TR + 1)[None, :]
    np.minimum(tw, t1[:, None], out=tw)
    g = xneg[tw, jj[:, None]].max(1)
    starts = np.unique(p2, return_index=True)[1]       # every cand has self live
    segmax = np.maximum.reduceat(g, starts)
    keep = xvs >= segmax
    mm, tgs = mm[keep], tgs[keep]
    o = np.lexsort((mm, tgs))
    return tgs[o], mm[o].astype(np.int64)


_PROGRAM_CACHE = {}


def kernel(traces, channel_locations):
    traces = np.ascontiguousarray(np.asarray(traces, np.float32))
    xneg = -traces
    xb = xneg.astype(ml_dtypes.bfloat16)
    adj = _adjacency(channel_locations)
    nbr, nbr_ok = _nbr_table(adj)
    identity = np.eye(128, dtype=np.float32).astype(ml_dtypes.bfloat16)

    if "full" not in _PROGRAM_CACHE:
        _PROGRAM_CACHE["full"] = build_program()
    nc = _PROGRAM_CACHE["full"]

    starts = [min(max(c * INT - TR, 0), N - T_LOC) for c in range(NCORES)]
    in_maps = [{
        "xs": np.ascontiguousarray(xb[starts[c]: starts[c] + T_LOC]),
        "ident": identity,
    } for c in range(NCORES)]
    try:
        res = run_bass_kernel_spmd(nc, in_maps, list(range(NCORES)))
    except Exception:
        time.sleep(2.0)
        res = run_bass_kernel_spmd(nc, in_maps, list(range(NCORES)))
    results = res.results

    all_t, all_c = [], []
    for c in range(NCORES):
        out = np.asarray(results[c]["b16"]).reshape(128, 3, NB)
        Bf = out.transpose(1, 0, 2).reshape(384, NB).astype(np.float32)
        t_, c_ = _postprocess_core(Bf, xneg, xb, nbr, nbr_ok, starts[c],
                                   c * INT, (c + 1) * INT)
        all_t.append(t_)
        all_c.append(c_)

    times = np.concatenate(all_t) if all_t else np.empty(0, np.int64)
    chans = np.concatenate(all_c) if all_c else np.empty(0, np.int64)
    times, chans = times[:MAX_DET], chans[:MAX_DET]
    out_t = np.full(MAX_DET, -1, np.int64)
    out_c = np.full(MAX_DET, -1, np.int32)
    out_t[: times.size] = times
    out_c[: chans.size] = chans
    return out_t, out_c
